# revision 26
# baseline (speedup 1.0000x reference)
"""CrossTransformerBlock3D Trainium2 kernel, v3 (fp8 DoubleRow).

Shards D (32) into 8 slabs of 4 across 8 NeuronCores; each core runs the
full block on its slab (256 windows of 64 tokens, groups of 8 windows =
512 tokens) with no collectives.

v3 structure (vs v2):
  - all linears (q/k/v/proj/fc1/fc2) run fp8e4m3 DoubleRow matmuls:
    192-channel contraction folded [96, 2] (768 folded [128, 3, 2]),
    0.5 cycles/row on the PE. Weights scaled x64 host-side; descale is
    folded into the PSUM evictions (ACT activation scale / DVE ops).
  - attention in packed single-window tiles: per (u, head) the score psum
    [128, 64] holds window A keys on partitions 0-63 and window B keys on
    64-127, same query columns; no masked quadrants at all, exp volume
    halves. Bias lands via a DoubleRow preload matmul; exp adds ln(16) so
    attn is stored x16 in fp8 (cancelled by the rowsum reciprocal).
  - rowsums via one accumulated [12, 256] indicator matmul chain; the
    softmax normalize + x1/16 descale + fp8 cast is one DVE
    scalar_tensor_tensor over the 2-bank AV psum.
  - proj and fc2 outputs are computed TOKEN-major (lhsT = activations),
    so the residual add consumes them directly: no transpose-back.
  - LN rstd via ACT Ln+Exp (same act table family as attention's Exp:
    natural_log_exp) - only Gelu forces a table switch (2 per batch).
  - phase-major batches of 8 groups; LN applies on Pool; PSUM evictions
    split ACT/DVE (Pool cannot touch PSUM).
"""

import math
import numpy as np
import ml_dtypes

import concourse.bass as bass
import concourse.tile as tile
from concourse import bacc, mybir
from concourse.bass_utils import run_bass_kernel_spmd

F32 = mybir.dt.float32
BF16 = mybir.dt.bfloat16
F8 = mybir.dt.float8e4
AF = mybir.ActivationFunctionType
ALU = mybir.AluOpType
DR = mybir.MatmulPerfMode.DoubleRow
E4M3 = ml_dtypes.float8_e4m3

# Problem shape (hardcoded per contract)
B, D, H, W, C = 1, 32, 64, 64, 192
NH, HD = 6, 32
SCALE = HD ** -0.5
N_CORES = 8
DS = D // N_CORES            # 4 depth per core = one window depth
NWH, NWW = H // 4, W // 4    # 16 x 16 windows per core
N_WIN = NWH * NWW            # 256 windows/core
GROUP_WIN = 8                # windows per group (512 tokens)
N_GROUPS = N_WIN // GROUP_WIN  # 32
BATCH = 8                    # groups per phase-major batch
FFN = 4 * C                  # 768
WS8 = 64.0                   # fp8 weight scale
LN16 = math.log(16.0)        # attn stored x16 in fp8


def _rel_index():
    ws = (4, 4, 4)
    coords = np.stack(np.meshgrid(np.arange(ws[0]), np.arange(ws[1]), np.arange(ws[2]), indexing='ij'))
    cf = coords.reshape(3, -1)
    rel = (cf[:, :, None] - cf[:, None, :]).transpose(1, 2, 0).copy()
    rel[:, :, 0] += ws[0] - 1
    rel[:, :, 1] += ws[1] - 1
    rel[:, :, 2] += ws[2] - 1
    rel[:, :, 0] *= (2 * ws[1] - 1) * (2 * ws[2] - 1)
    rel[:, :, 1] *= 2 * ws[2] - 1
    return rel.sum(-1)


def bf16(a):
    return np.asarray(a, np.float32).astype(ml_dtypes.bfloat16)


def f8(a):
    return np.asarray(a, np.float32).astype(E4M3)


def fold96(w):
    """[192, C'] -> [96, 2, C'] DoubleRow fold (chan c at [c%96, c//96])."""
    return np.ascontiguousarray(w.reshape(2, 96, w.shape[-1]).transpose(1, 0, 2))


def win_permute(slab):
    """[DS,H,W,C] -> [N_WIN*64, C] in (wh, ww, d, i, j) token order."""
    t = slab.reshape(DS, NWH, 4, NWW, 4, C).transpose(1, 3, 0, 2, 4, 5)
    return np.ascontiguousarray(t.reshape(N_WIN * 64, C))


def win_unpermute(flat):
    """[N_WIN*64, C] -> [DS,H,W,C]."""
    t = flat.reshape(NWH, NWW, DS, 4, 4, C).transpose(2, 0, 3, 1, 4, 5)
    return np.ascontiguousarray(t.reshape(DS, H, W, C))


def build_program(weights_np=None):
    nc = bacc.Bacc("TRN2", target_bir_lowering=False, debug=False)

    xs = nc.dram_tensor("xs", [N_WIN * 64, C], F32, kind="ExternalInput").ap()
    ys = nc.dram_tensor("ys", [N_WIN * 64, C], F32, kind="ExternalInput").ap()
    wq8 = nc.dram_tensor("wq8", [96, 2, C], F8, kind="ExternalInput").ap()
    wk8 = nc.dram_tensor("wk8", [96, 2, C], F8, kind="ExternalInput").ap()
    wv8 = nc.dram_tensor("wv8", [96, 2, C], F8, kind="ExternalInput").ap()
    wp8 = nc.dram_tensor("wp8", [96, 2, C], F8, kind="ExternalInput").ap()
    w18 = nc.dram_tensor("w18", [96, 2, FFN], F8, kind="ExternalInput").ap()
    w28 = nc.dram_tensor("w28", [128, 3, 2, C], F8, kind="ExternalInput").ap()
    # bias preload lhsT: [64, 2, h, 128]; sub 1 is zero (pairs with idn28)
    bias2T8 = nc.dram_tensor("bias2T8", [64, 2, NH, 128], F8, kind="ExternalInput").ap()
    idn28 = nc.dram_tensor("idn28", [64, 2, 64], F8, kind="ExternalInput").ap()
    # rowsum router: [k, h, r] = 1 iff r == 3*(2*(h//3) + (k//64)) + h%3
    ehAB8 = nc.dram_tensor("ehAB8", [128, NH, 12], F8, kind="ExternalInput").ap()
    # rbc broadcast lhsT: eB[k, (s,w), 32m+i] = (k == 3*(2s+w)+m)
    eB = nc.dram_tensor("eB", [12, 4, 96], BF16, kind="ExternalInput").ap()
    identB = nc.dram_tensor("identB", [128, 128], BF16, kind="ExternalInput").ap()
    out = nc.dram_tensor("out", [N_WIN * 64, C], F32, kind="ExternalOutput").ap()

    with tile.TileContext(nc) as tc:
        kernel_body(tc, xs, ys, wq8, wk8, wv8, wp8, w18, w28, bias2T8, idn28,
                    ehAB8, eB, identB, out)
    nc.compile()
    return nc


def kernel_body(tc, xs, ys, wq8, wk8, wv8, wp8, w18, w28, bias2T8, idn28,
                ehAB8, eB, identB, out):
    nc = tc.nc
    ctx_pools = []

    def pool(name, bufs, space="SBUF"):
        p = tc.tile_pool(name=name, bufs=bufs, space=space)
        ctx_pools.append(p)
        return p.__enter__()

    singles = pool("singles", 1)
    sb2 = pool("sb2", 2)       # transient within one phase
    sbG = pool("sbG", BATCH)   # live across one phase boundary
    sb3 = pool("sb3", 3)
    # PSUM: 8 banks. tp 2x1 + ps4 4x1 + ao 1x2 = 8 banks.
    ps_tp = pool("ps_tp", 2, space="PSUM")
    ps4 = pool("ps4", 4, space="PSUM")
    ps_ao = pool("ps_ao", 1, space="PSUM")

    def load_const(name, src_ap, shape, dtype):
        t = singles.tile(shape, dtype, tag=name)
        nc.sync.dma_start(out=t, in_=src_ap)
        return t

    wq_sb = load_const("wq_sb", wq8, [96, 2, C], F8)
    wk_sb = load_const("wk_sb", wk8, [96, 2, C], F8)
    wv_sb = load_const("wv_sb", wv8, [96, 2, C], F8)
    wp_sb = load_const("wp_sb", wp8, [96, 2, C], F8)
    w1_sb = load_const("w1_sb", w18, [96, 2, FFN], F8)
    w2_sb = load_const("w2_sb", w28, [128, 3, 2, C], F8)
    bias_sb = load_const("bias_sb", bias2T8, [64, 2, NH, 128], F8)
    idn2_sb = load_const("idn2_sb", idn28, [64, 2, 64], F8)
    eh_sb = load_const("eh_sb", ehAB8, [128, NH, 12], F8)
    eB_sb = load_const("eB_sb", eB, [12, 4, 96], BF16)
    idB_sb = load_const("idB_sb", identB, [128, 128], BF16)
    eps_sb = singles.tile([128, 1], F32, tag="eps")
    nc.vector.memset(eps_sb, 1e-5)
    ln16_sb = singles.tile([128, 1], F32, tag="ln16")
    nc.vector.memset(ln16_sb, LN16)

    state = {}

    def ln_stats(src_f32, mv_all, idx):
        st = sb3.tile([128, 6], F32, tag="ln_st")
        nc.vector.bn_stats(out=st, in_=src_f32)
        nc.vector.bn_aggr(out=mv_all[:, :, idx], in_=st)

    def ln_finalize_batch(mvb, half=None):
        """var -> 1/sqrt(var+eps) in-place: one Ln + one Exp on ACT.
        half=0/1 finalizes only that half of the batch dim."""
        hb = BATCH // 2
        sl = slice(None) if half is None else slice(half * hb, half * hb + hb)
        v = mvb[:, sl, 1, :]
        nc.scalar.activation(out=v, in_=v, func=AF.Ln, bias=eps_sb, scale=1.0)
        nc.scalar.activation(out=v, in_=v, func=AF.Exp, scale=-0.5)

    def transposed_fold(keep, mv, base, dst_tag, evict_eng):
        """[128,4,C] f32 + stats -> [96, 2, 512] fp8 feature-major fold."""
        tp = ps_tp.tile([96, 4, 2, 128], BF16, tag="tp")
        for u in range(4):
            xn_t = sb3.tile([128, C], BF16, tag="xn_t")
            nc.gpsimd.tensor_scalar(out=xn_t, in0=keep[:, u, :],
                                    scalar1=mv[:, 0, base + u:base + u + 1],
                                    scalar2=mv[:, 1, base + u:base + u + 1],
                                    op0=ALU.subtract, op1=ALU.mult)
            nc.tensor.transpose(tp[:, u, 0, :], xn_t[:, 0:96], idB_sb)
            nc.tensor.transpose(tp[:, u, 1, :], xn_t[:, 96:192], idB_sb)
        dst_t = sbG.tile([96, 2, 512], F8, tag=dst_tag)
        dst = dst_t.rearrange("p s (u q) -> p s u q", u=4)
        src = tp.rearrange("p u c q -> p c u q")
        if evict_eng == "act":
            nc.scalar.activation(out=dst, in_=src, func=AF.Copy)
        else:
            nc.vector.tensor_copy(out=dst, in_=src)
        return dst_t

    def linear_qk(dst_tag, w_sb_t, rhs):
        """feature-major [C, 512] bf16 pair via 2 DR matmuls.
        hi evicts on ACT, lo on DVE (x 1/WS8 descale)."""
        p_hi = ps4.tile([128, 512], F32, tag="p")
        nc.tensor.matmul(p_hi, w_sb_t[:, :, 0:128], rhs, start=True, stop=True,
                         perf_mode=DR)
        d_hi = sbG.tile([128, 512], BF16, tag=dst_tag + "_hi")
        nc.scalar.activation(out=d_hi, in_=p_hi, func=AF.Copy, scale=1.0 / WS8)
        p_lo = ps4.tile([128, 512], F32, tag="p")
        p_lo = p_lo[0:64, :]
        nc.tensor.matmul(p_lo, w_sb_t[:, :, 128:192], rhs, start=True, stop=True,
                         perf_mode=DR)
        d_lo = sbG.tile([64, 512], BF16, tag=dst_tag + "_lo")
        nc.vector.tensor_scalar(out=d_lo, in0=p_lo, scalar1=1.0 / WS8,
                                scalar2=None, op0=ALU.mult)
        return d_hi, d_lo

    def load1(g):
        x_keep = sbG.tile([128, 4, C], F32, tag="x_keep")
        nc.sync.dma_start(
            out=x_keep,
            in_=xs[g * 512:(g + 1) * 512, :].rearrange("(u p) c -> p u c", u=4))
        y_keep = sbG.tile([128, 4, C], F32, tag="y_keep")
        nc.sync.dma_start(
            out=y_keep,
            in_=ys[g * 512:(g + 1) * 512, :].rearrange("(u p) c -> p u c", u=4))
        state[g] = dict(x_keep=x_keep, y_keep=y_keep)

    def stats1(g, mvb):
        gi = g % BATCH
        st = state[g]
        for u in range(4):
            ln_stats(st["x_keep"][:, u, :], mvb[:, gi], u)
            ln_stats(st["y_keep"][:, u, :], mvb[:, gi], 4 + u)

    def phase1b(g, mvb):
        gi = g % BATCH
        st = state[g]
        x_keep, y_keep = st["x_keep"], st["y_keep"]
        mv = mvb[:, gi]

        xnT = transposed_fold(x_keep, mv, 0, "xnT", "act")
        ynT = transposed_fold(y_keep, mv, 4, "ynT", "dve")

        q_hi, q_lo = linear_qk("q", wq_sb, ynT)
        k_hi, k_lo = linear_qk("k", wk_sb, xnT)

        # v token-major [128, 4, 192] fp8 (x16): DR mms in u-pairs
        v8 = sbG.tile([128, 4, C], F8, tag="v8")
        for up in range(2):
            pv = ps4.tile([128, 512], F32, tag="p")
            for i in range(2):
                u = 2 * up + i
                nc.tensor.matmul(pv[:, i * C:(i + 1) * C],
                                 xnT[:, :, 128 * u:128 * u + 128], wv_sb,
                                 start=True, stop=True, perf_mode=DR)
            nc.scalar.activation(
                out=v8[:, 2 * up:2 * up + 2, :],
                in_=pv[:, 0:2 * C].rearrange("p (i c) -> p i c", i=2),
                func=AF.Copy, scale=1.0 / 4.0)
        st.update(q_hi=q_hi, q_lo=q_lo, k_hi=k_hi, k_lo=k_lo, v8=v8)
        del st["y_keep"]

    def phase2(g):
        st = state[g]
        q_hi, q_lo = st["q_hi"], st["q_lo"]
        k_hi, k_lo = st["k_hi"], st["k_lo"]
        v8 = st["v8"]

        attn8 = sb2.tile([128, 4, NH, 64], F8, tag="attn8")
        for u in range(4):
            sc = ps4.tile([128, NH, 64], F32, tag="p", name=f"sc_{g}_{u}")
            for h in range(NH):
                if h < 4:
                    k_sl, q_sl, off = k_hi, q_hi, 32 * h
                else:
                    k_sl, q_sl, off = k_lo, q_lo, 32 * (h - 4)
                nc.tensor.matmul(sc[:, h, :], bias_sb[:, :, h, :], idn2_sb,
                                 start=True, stop=False, perf_mode=DR)
                c0 = 128 * u
                nc.tensor.matmul(
                    sc[0:64, h, :],
                    k_sl[off:off + 32, c0:c0 + 64],
                    q_sl[off:off + 32, c0:c0 + 64],
                    start=False, stop=False,
                    tile_position=(off, 0))
                nc.tensor.matmul(
                    sc[64:128, h, :],
                    k_sl[off:off + 32, c0 + 64:c0 + 128],
                    q_sl[off:off + 32, c0 + 64:c0 + 128],
                    start=False, stop=True,
                    tile_position=(off, 64))
            nc.scalar.activation(out=attn8[:, u, :, :],
                                 in_=sc.rearrange("p h j -> p (h j)"),
                                 func=AF.Exp, bias=ln16_sb)

        # rowsums -> [12, 256] (row 4m+2s+w), one accumulated chain
        r12 = ps4.tile([12, 256], F32, tag="p", name=f"r12_{g}")
        for h in range(NH):
            nc.tensor.matmul(r12.rearrange("p (u j) -> p u j", u=4),
                             eh_sb[:, h, :], attn8[:, :, h, :],
                             start=(h == 0), stop=(h == NH - 1))
        r12_sb = sb3.tile([12, 256], BF16, tag="r12_sb")
        with nc.allow_low_precision(reason="softmax 1/sum in bf16"):
            nc.vector.reciprocal(out=r12_sb, in_=r12)
        rbc = [sb3.tile([96, 2, 4, 64], BF16, tag=f"rbc{w}", name=f"rbc{w}_{g}")
               for w in range(2)]
        for s in range(2):
            for w in range(2):
                r0 = 3 * (2 * s + w)
                nc.sync.dma_start(
                    out=rbc[w][:, s, :, :],
                    in_=r12_sb[r0:r0 + 3, :].unsqueeze(1)
                        .broadcast_to([3, 32, 256]))

        # AV into [96, 2, 512] 2-bank psum; head h=(s,m) at rows 32m, sub s
        ao = ps_ao.tile([96, 2, 512], F32, tag="ao")
        for u in range(4):
            for h in range(NH):
                s, m = h // 3, h % 3
                for w in range(2):
                    nc.tensor.matmul(
                        ao[32 * m:32 * m + 32, s,
                           128 * u + 64 * w:128 * u + 64 * w + 64],
                        v8[64 * w:64 * w + 64, u, 32 * h:32 * h + 32],
                        attn8[64 * w:64 * w + 64, u, h, :],
                        start=True, stop=True,
                        tile_position=(64 * w, 32 * m))
        aoT8 = sbG.tile([96, 2, 512], F8, tag="aoT8")
        ao_v = ao.rearrange("p s (u w j) -> p s u w j", u=4, w=2)
        aoT_v = aoT8.rearrange("p s (u w j) -> p s u w j", u=4, w=2)
        with nc.allow_low_precision(reason="attn out fp8"):
            for w in range(2):
                nc.vector.scalar_tensor_tensor(
                    out=aoT_v[:, :, :, w, :], in0=ao_v[:, :, :, w, :],
                    scalar=1.0 / 16.0, in1=rbc[w],
                    op0=ALU.mult, op1=ALU.mult)
        st["aoT8"] = aoT8
        for kk in ("q_hi", "q_lo", "k_hi", "k_lo", "v8"):
            del st[kk]

    def phase3a(g, mvb2):
        gi = g % BATCH
        st = state[g]
        aoT8 = st["aoT8"]
        x_keep = st["x_keep"]
        # proj TOKEN-major: out[tok, c] per u-pair psum, fused residual
        x2 = sbG.tile([128, 4, C], F32, tag="x2")
        for up in range(2):
            pp = ps4.tile([128, 512], F32, tag="p")
            for i in range(2):
                u = 2 * up + i
                nc.tensor.matmul(pp[:, i * C:(i + 1) * C],
                                 aoT8[:, :, 128 * u:128 * u + 128], wp_sb,
                                 start=True, stop=True, perf_mode=DR)
            nc.vector.scalar_tensor_tensor(
                out=x2[:, 2 * up:2 * up + 2, :],
                in0=pp[:, 0:2 * C].rearrange("p (i c) -> p i c", i=2),
                scalar=1.0 / WS8,
                in1=x_keep[:, 2 * up:2 * up + 2, :],
                op0=ALU.mult, op1=ALU.add)
        for u in range(4):
            ln_stats(x2[:, u, :], mvb2[:, gi], u)
        st["x2"] = x2
        for kk in ("x_keep", "aoT8"):
            del st[kk]

    def phase3b(g, mvb2):
        gi = g % BATCH
        st = state[g]
        xn2T = transposed_fold(st["x2"], mvb2[:, gi], 0, "xn2T", "dve")
        st["xn2T"] = xn2T

    def phase4(g):
        st = state[g]
        xn2T = st["xn2T"]
        hT = sb2.tile([128, 6, 512], F8, tag="hT")
        for m in range(6):
            ph = ps4.tile([128, 512], F32, tag="p")
            nc.tensor.matmul(ph, w1_sb[:, :, 128 * m:128 * m + 128], xn2T,
                             start=True, stop=True, perf_mode=DR)
            nc.scalar.activation(out=hT[:, m, :], in_=ph, func=AF.Gelu,
                                 scale=1.0 / WS8)

        x2 = st["x2"]
        o_t = sb2.tile([128, 4, C], F32, tag="o_t")
        for up in range(2):
            p2 = ps4.tile([128, 512], F32, tag="p")
            for i in range(2):
                u = 2 * up + i
                for j in range(3):
                    nc.tensor.matmul(
                        p2[:, i * C:(i + 1) * C],
                        hT[:, 2 * j:2 * j + 2, 128 * u:128 * u + 128],
                        w2_sb[:, j, :, :],
                        start=(j == 0), stop=(j == 2), perf_mode=DR)
            nc.vector.scalar_tensor_tensor(
                out=o_t[:, 2 * up:2 * up + 2, :],
                in0=p2[:, 0:2 * C].rearrange("p (i c) -> p i c", i=2),
                scalar=1.0 / WS8,
                in1=x2[:, 2 * up:2 * up + 2, :],
                op0=ALU.mult, op1=ALU.add)
        nc.sync.dma_start(
            out=out[g * 512:(g + 1) * 512, :].rearrange("(u p) c -> p u c", u=4),
            in_=o_t)
        del state[g]

    sbB = pool("sbB", 2)
    with nc.allow_low_precision(reason="fp8 kernel"):
        mvb_cur = None
        for b0 in range(0, N_GROUPS, BATCH):
            gs = range(b0, b0 + BATCH)
            last = b0 + BATCH >= N_GROUPS
            if b0 == 0:
                mvb_cur = sbB.tile([128, BATCH, 2, 8], F32, tag="mvb",
                                   name="mvb_0")
                for g in gs:
                    load1(g)
                for g in gs:
                    stats1(g, mvb_cur)
            mvb_this = mvb_cur
            mvb2 = sbB.tile([128, BATCH, 2, 4], F32, tag="mvb2")
            hb = BATCH // 2
            ln_finalize_batch(mvb_this, 0)
            for g in list(gs)[:hb]:
                phase1b(g, mvb_this)
            ln_finalize_batch(mvb_this, 1)
            for g in list(gs)[hb:]:
                phase1b(g, mvb_this)
            for g in gs:
                phase2(g)
            for g in gs:
                phase3a(g, mvb2)
            if not last:
                # prefetch next batch's x/y while P3b/P4 of this batch run
                for g in gs:
                    load1(g + BATCH)
                mvb_cur = sbB.tile([128, BATCH, 2, 8], F32, tag="mvb",
                                   name=f"mvb_{b0 + BATCH}")
            ln_finalize_batch(mvb2, 0)
            for g in list(gs)[:hb]:
                phase3b(g, mvb2)
            ln_finalize_batch(mvb2, 1)
            for g in list(gs)[hb:]:
                phase3b(g, mvb2)
            for g in gs:
                phase4(g)
                if not last:
                    stats1(g + BATCH, mvb_cur)

    for p in reversed(ctx_pools):
        p.__exit__(None, None, None)


def prep_inputs(inputs):
    """Host-side prep: fold norms/scales into weights, build constants."""
    f32 = lambda a: np.ascontiguousarray(np.asarray(a, np.float32))
    x, y = f32(inputs['x']), f32(inputs['y'])
    qkv_w, qkv_b = f32(inputs['qkv_w']), f32(inputs['qkv_b'])
    g1, b1n = f32(inputs['norm1_g']), f32(inputs['norm1_b'])
    g2, b2n = f32(inputs['norm2_g']), f32(inputs['norm2_b'])

    wq_eff = g1[:, None] * qkv_w[:, 0:C] * SCALE
    wk_eff = g1[:, None] * qkv_w[:, C:2 * C]
    wv_eff = g1[:, None] * qkv_w[:, 2 * C:]
    bq = b1n @ qkv_w[:, 0:C] * SCALE + qkv_b[0:C] * SCALE
    bk = b1n @ qkv_w[:, C:2 * C] + qkv_b[C:2 * C]
    bv = b1n @ qkv_w[:, 2 * C:] + qkv_b[2 * C:]
    w1_eff = g2[:, None] * f32(inputs['fc1_w'])
    b1_eff = b2n @ f32(inputs['fc1_w']) + f32(inputs['fc1_b'])
    assert not (np.any(bq) or np.any(bk) or np.any(bv) or np.any(b1_eff) or
                np.any(f32(inputs['proj_b'])) or np.any(f32(inputs['fc2_b']))), \
        "nonzero biases not folded in this build"

    rel = _rel_index()
    rpb = f32(inputs['rpb_table'])
    bias_full = rpb[rel]                     # [n(query), m(key), NH]

    # preload: out[p, j] = sum_{m,s} lhsT[m, s, p] * idn2[m, s, j]
    #                    = WS8*bias[j, k(p)] * (1/WS8) = bias_full[j, p%64]
    b2t = np.zeros((64, 2, NH, 128), np.float32)
    for h in range(NH):
        bh = bias_full[:, :, h]              # [query j, key k]
        b2t[:, 0, h, 0:64] = WS8 * bh        # lhsT[j, 0, h, k]
        b2t[:, 0, h, 64:128] = WS8 * bh

    idn2 = np.zeros((64, 2, 64), np.float32)
    idn2[:, 0, :] = np.eye(64, dtype=np.float32) / WS8

    ehab = np.zeros((128, NH, 12), np.float32)
    for h in range(NH):
        s, m = h // 3, h % 3
        for w in range(2):
            ehab[64 * w:64 * w + 64, h, 3 * (2 * s + w) + m] = 1.0

    ident = np.eye(128, dtype=np.float32)

    ebm = np.zeros((12, 4, 96), np.float32)
    for s in range(2):
        for w in range(2):
            for m in range(3):
                ebm[3 * (2 * s + w) + m, 2 * s + w, 32 * m:32 * m + 32] = 1.0

    shared = {
        'wq8': f8(fold96(wq_eff * WS8)),
        'wk8': f8(fold96(wk_eff * WS8)),
        'wv8': f8(fold96(wv_eff * WS8)),
        'wp8': f8(fold96(f32(inputs['proj_w']) * WS8)),
        'w18': f8(fold96(w1_eff * WS8)),
        'w28': f8(np.ascontiguousarray(
            (f32(inputs['fc2_w']) * WS8).reshape(3, 2, 128, C)
            .transpose(2, 0, 1, 3))),
        'bias2T8': f8(b2t),
        'idn28': f8(idn2),
        'ehAB8': f8(ehab),
        'eB': bf16(ebm),
        'identB': bf16(ident),
    }

    in_maps = []
    for i in range(N_CORES):
        m = dict(shared)
        m['xs'] = win_permute(x[0, i * DS:(i + 1) * DS])
        m['ys'] = win_permute(y[0, i * DS:(i + 1) * DS])
        in_maps.append(m)
    return in_maps


_CACHED_NC = None


def get_program(in_maps=None):
    global _CACHED_NC
    if _CACHED_NC is None:
        _CACHED_NC = build_program()
    return _CACHED_NC


def kernel(**inputs):
    in_maps = prep_inputs(inputs)
    nc = get_program(in_maps)
    res = run_bass_kernel_spmd(nc, in_maps, list(range(N_CORES)))
    outs = [win_unpermute(res.results[i]["out"]) for i in range(N_CORES)]
    full = np.concatenate([o[None] for o in outs], axis=0)  # [8, DS, H, W, C]
    full = full.reshape(1, D, H, W, C).astype(np.float32)
    return full


# revision 27
# speedup vs baseline: 1.0001x; 1.0001x over previous
"""CrossTransformerBlock3D Trainium2 kernel, v3 (fp8 DoubleRow).

Shards D (32) into 8 slabs of 4 across 8 NeuronCores; each core runs the
full block on its slab (256 windows of 64 tokens, groups of 8 windows =
512 tokens) with no collectives.

v3 structure (vs v2):
  - all linears (q/k/v/proj/fc1/fc2) run fp8e4m3 DoubleRow matmuls:
    192-channel contraction folded [96, 2] (768 folded [128, 3, 2]),
    0.5 cycles/row on the PE. Weights scaled x64 host-side; descale is
    folded into the PSUM evictions (ACT activation scale / DVE ops).
  - attention in packed single-window tiles: per (u, head) the score psum
    [128, 64] holds window A keys on partitions 0-63 and window B keys on
    64-127, same query columns; no masked quadrants at all, exp volume
    halves. Bias lands via a DoubleRow preload matmul; exp adds ln(16) so
    attn is stored x16 in fp8 (cancelled by the rowsum reciprocal).
  - rowsums via one accumulated [12, 256] indicator matmul chain; the
    softmax normalize + x1/16 descale + fp8 cast is one DVE
    scalar_tensor_tensor over the 2-bank AV psum.
  - proj and fc2 outputs are computed TOKEN-major (lhsT = activations),
    so the residual add consumes them directly: no transpose-back.
  - LN rstd via ACT Ln+Exp (same act table family as attention's Exp:
    natural_log_exp) - only Gelu forces a table switch (2 per batch).
  - phase-major batches of 8 groups; LN applies on Pool; PSUM evictions
    split ACT/DVE (Pool cannot touch PSUM).
"""

import math
import numpy as np
import ml_dtypes

import concourse.bass as bass
import concourse.tile as tile
from concourse import bacc, mybir
from concourse.bass_utils import run_bass_kernel_spmd

F32 = mybir.dt.float32
BF16 = mybir.dt.bfloat16
F8 = mybir.dt.float8e4
AF = mybir.ActivationFunctionType
ALU = mybir.AluOpType
DR = mybir.MatmulPerfMode.DoubleRow
E4M3 = ml_dtypes.float8_e4m3

# Problem shape (hardcoded per contract)
B, D, H, W, C = 1, 32, 64, 64, 192
NH, HD = 6, 32
SCALE = HD ** -0.5
N_CORES = 8
DS = D // N_CORES            # 4 depth per core = one window depth
NWH, NWW = H // 4, W // 4    # 16 x 16 windows per core
N_WIN = NWH * NWW            # 256 windows/core
GROUP_WIN = 8                # windows per group (512 tokens)
N_GROUPS = N_WIN // GROUP_WIN  # 32
BATCH = 8                    # groups per phase-major batch
FFN = 4 * C                  # 768
WS8 = 64.0                   # fp8 weight scale
LN16 = math.log(16.0)        # attn stored x16 in fp8


def _rel_index():
    ws = (4, 4, 4)
    coords = np.stack(np.meshgrid(np.arange(ws[0]), np.arange(ws[1]), np.arange(ws[2]), indexing='ij'))
    cf = coords.reshape(3, -1)
    rel = (cf[:, :, None] - cf[:, None, :]).transpose(1, 2, 0).copy()
    rel[:, :, 0] += ws[0] - 1
    rel[:, :, 1] += ws[1] - 1
    rel[:, :, 2] += ws[2] - 1
    rel[:, :, 0] *= (2 * ws[1] - 1) * (2 * ws[2] - 1)
    rel[:, :, 1] *= 2 * ws[2] - 1
    return rel.sum(-1)


def bf16(a):
    return np.asarray(a, np.float32).astype(ml_dtypes.bfloat16)


def f8(a):
    return np.asarray(a, np.float32).astype(E4M3)


def fold96(w):
    """[192, C'] -> [96, 2, C'] DoubleRow fold (chan c at [c%96, c//96])."""
    return np.ascontiguousarray(w.reshape(2, 96, w.shape[-1]).transpose(1, 0, 2))


def win_permute(slab):
    """[DS,H,W,C] -> [N_WIN*64, C] in (wh, ww, d, i, j) token order."""
    t = slab.reshape(DS, NWH, 4, NWW, 4, C).transpose(1, 3, 0, 2, 4, 5)
    return np.ascontiguousarray(t.reshape(N_WIN * 64, C))


def win_unpermute(flat):
    """[N_WIN*64, C] -> [DS,H,W,C]."""
    t = flat.reshape(NWH, NWW, DS, 4, 4, C).transpose(2, 0, 3, 1, 4, 5)
    return np.ascontiguousarray(t.reshape(DS, H, W, C))


def build_program(weights_np=None):
    nc = bacc.Bacc("TRN2", target_bir_lowering=False, debug=False)

    xs = nc.dram_tensor("xs", [N_WIN * 64, C], F32, kind="ExternalInput").ap()
    ys = nc.dram_tensor("ys", [N_WIN * 64, C], F32, kind="ExternalInput").ap()
    wq8 = nc.dram_tensor("wq8", [96, 2, C], F8, kind="ExternalInput").ap()
    wk8 = nc.dram_tensor("wk8", [96, 2, C], F8, kind="ExternalInput").ap()
    wv8 = nc.dram_tensor("wv8", [96, 2, C], F8, kind="ExternalInput").ap()
    wp8 = nc.dram_tensor("wp8", [96, 2, C], F8, kind="ExternalInput").ap()
    w18 = nc.dram_tensor("w18", [96, 2, FFN], F8, kind="ExternalInput").ap()
    w28 = nc.dram_tensor("w28", [128, 3, 2, C], F8, kind="ExternalInput").ap()
    # bias preload: out[k, (h, j)] = idf.T @ biasR = bias_full[j, k%64, h]
    biasR8 = nc.dram_tensor("biasR8", [64, 2, NH, 64], F8, kind="ExternalInput").ap()
    idf8 = nc.dram_tensor("idf8", [64, 2, 128], F8, kind="ExternalInput").ap()
    # rowsum router: [k, h, r] = 1 iff r == 3*(2*(h//3) + (k//64)) + h%3
    ehAB8 = nc.dram_tensor("ehAB8", [128, NH, 12], F8, kind="ExternalInput").ap()
    # rbc broadcast lhsT: eB[k, (s,w), 32m+i] = (k == 3*(2s+w)+m)
    eB = nc.dram_tensor("eB", [12, 4, 96], BF16, kind="ExternalInput").ap()
    identB = nc.dram_tensor("identB", [128, 128], BF16, kind="ExternalInput").ap()
    out = nc.dram_tensor("out", [N_WIN * 64, C], F32, kind="ExternalOutput").ap()

    with tile.TileContext(nc) as tc:
        kernel_body(tc, xs, ys, wq8, wk8, wv8, wp8, w18, w28, biasR8, idf8,
                    ehAB8, eB, identB, out)
    nc.compile()
    return nc


def kernel_body(tc, xs, ys, wq8, wk8, wv8, wp8, w18, w28, biasR8, idf8,
                ehAB8, eB, identB, out):
    nc = tc.nc
    ctx_pools = []

    def pool(name, bufs, space="SBUF"):
        p = tc.tile_pool(name=name, bufs=bufs, space=space)
        ctx_pools.append(p)
        return p.__enter__()

    singles = pool("singles", 1)
    sb2 = pool("sb2", 2)       # transient within one phase
    sbG = pool("sbG", BATCH)   # live across one phase boundary
    sb3 = pool("sb3", 3)
    # PSUM: 8 banks. tp 2x1 + ps4 4x1 + ao 1x2 = 8 banks.
    ps_tp = pool("ps_tp", 2, space="PSUM")
    ps4 = pool("ps4", 4, space="PSUM")
    ps_ao = pool("ps_ao", 1, space="PSUM")

    def load_const(name, src_ap, shape, dtype):
        t = singles.tile(shape, dtype, tag=name)
        nc.sync.dma_start(out=t, in_=src_ap)
        return t

    wq_sb = load_const("wq_sb", wq8, [96, 2, C], F8)
    wk_sb = load_const("wk_sb", wk8, [96, 2, C], F8)
    wv_sb = load_const("wv_sb", wv8, [96, 2, C], F8)
    wp_sb = load_const("wp_sb", wp8, [96, 2, C], F8)
    w1_sb = load_const("w1_sb", w18, [96, 2, FFN], F8)
    w2_sb = load_const("w2_sb", w28, [128, 3, 2, C], F8)
    bias_sb = load_const("bias_sb", biasR8, [64, 2, NH, 64], F8)
    idf_sb = load_const("idf_sb", idf8, [64, 2, 128], F8)
    eh_sb = load_const("eh_sb", ehAB8, [128, NH, 12], F8)
    eB_sb = load_const("eB_sb", eB, [12, 4, 96], BF16)
    idB_sb = load_const("idB_sb", identB, [128, 128], BF16)
    eps_sb = singles.tile([128, 1], F32, tag="eps")
    nc.vector.memset(eps_sb, 1e-5)
    ln16_sb = singles.tile([128, 1], F32, tag="ln16")
    nc.vector.memset(ln16_sb, LN16)

    state = {}

    def ln_stats(src_f32, mv_all, idx):
        st = sb3.tile([128, 6], F32, tag="ln_st")
        nc.vector.bn_stats(out=st, in_=src_f32)
        nc.vector.bn_aggr(out=mv_all[:, :, idx], in_=st)

    def ln_finalize_batch(mvb, half=None):
        """var -> 1/sqrt(var+eps) in-place: one Ln + one Exp on ACT.
        half=0/1 finalizes only that half of the batch dim."""
        hb = BATCH // 2
        sl = slice(None) if half is None else slice(half * hb, half * hb + hb)
        v = mvb[:, sl, 1, :]
        nc.scalar.activation(out=v, in_=v, func=AF.Ln, bias=eps_sb, scale=1.0)
        nc.scalar.activation(out=v, in_=v, func=AF.Exp, scale=-0.5)

    def transposed_fold(keep, mv, base, dst_tag, evict_eng):
        """[128,4,C] f32 + stats -> [96, 2, 512] fp8 feature-major fold."""
        tp = ps_tp.tile([96, 4, 2, 128], BF16, tag="tp")
        for u in range(4):
            xn_t = sb3.tile([128, C], BF16, tag="xn_t")
            nc.gpsimd.tensor_scalar(out=xn_t, in0=keep[:, u, :],
                                    scalar1=mv[:, 0, base + u:base + u + 1],
                                    scalar2=mv[:, 1, base + u:base + u + 1],
                                    op0=ALU.subtract, op1=ALU.mult)
            nc.tensor.transpose(tp[:, u, 0, :], xn_t[:, 0:96], idB_sb)
            nc.tensor.transpose(tp[:, u, 1, :], xn_t[:, 96:192], idB_sb)
        dst_t = sbG.tile([96, 2, 512], F8, tag=dst_tag)
        dst = dst_t.rearrange("p s (u q) -> p s u q", u=4)
        src = tp.rearrange("p u c q -> p c u q")
        if evict_eng == "act":
            nc.scalar.activation(out=dst, in_=src, func=AF.Copy)
        else:
            nc.vector.tensor_copy(out=dst, in_=src)
        return dst_t

    def linear_qk(dst_tag, w_sb_t, rhs):
        """feature-major [C, 512] bf16 pair via 2 DR matmuls.
        hi evicts on ACT, lo on DVE (x 1/WS8 descale)."""
        p_hi = ps4.tile([128, 512], F32, tag="p")
        nc.tensor.matmul(p_hi, w_sb_t[:, :, 0:128], rhs, start=True, stop=True,
                         perf_mode=DR)
        d_hi = sbG.tile([128, 512], BF16, tag=dst_tag + "_hi")
        nc.scalar.activation(out=d_hi, in_=p_hi, func=AF.Copy, scale=1.0 / WS8)
        p_lo = ps4.tile([128, 512], F32, tag="p")
        p_lo = p_lo[0:64, :]
        nc.tensor.matmul(p_lo, w_sb_t[:, :, 128:192], rhs, start=True, stop=True,
                         perf_mode=DR)
        d_lo = sbG.tile([64, 512], BF16, tag=dst_tag + "_lo")
        nc.vector.tensor_scalar(out=d_lo, in0=p_lo, scalar1=1.0 / WS8,
                                scalar2=None, op0=ALU.mult)
        return d_hi, d_lo

    def load1(g):
        x_keep = sbG.tile([128, 4, C], F32, tag="x_keep")
        nc.sync.dma_start(
            out=x_keep,
            in_=xs[g * 512:(g + 1) * 512, :].rearrange("(u p) c -> p u c", u=4))
        y_keep = sbG.tile([128, 4, C], F32, tag="y_keep")
        nc.sync.dma_start(
            out=y_keep,
            in_=ys[g * 512:(g + 1) * 512, :].rearrange("(u p) c -> p u c", u=4))
        state[g] = dict(x_keep=x_keep, y_keep=y_keep)

    def stats1(g, mvb):
        gi = g % BATCH
        st = state[g]
        for u in range(4):
            ln_stats(st["x_keep"][:, u, :], mvb[:, gi], u)
            ln_stats(st["y_keep"][:, u, :], mvb[:, gi], 4 + u)

    def phase1b(g, mvb):
        gi = g % BATCH
        st = state[g]
        x_keep, y_keep = st["x_keep"], st["y_keep"]
        mv = mvb[:, gi]

        xnT = transposed_fold(x_keep, mv, 0, "xnT", "act")
        ynT = transposed_fold(y_keep, mv, 4, "ynT", "dve")

        q_hi, q_lo = linear_qk("q", wq_sb, ynT)
        k_hi, k_lo = linear_qk("k", wk_sb, xnT)

        # v token-major [128, 4, 192] fp8 (x16): DR mms in u-pairs
        v8 = sbG.tile([128, 4, C], F8, tag="v8")
        for up in range(2):
            pv = ps4.tile([128, 512], F32, tag="p")
            for i in range(2):
                u = 2 * up + i
                nc.tensor.matmul(pv[:, i * C:(i + 1) * C],
                                 xnT[:, :, 128 * u:128 * u + 128], wv_sb,
                                 start=True, stop=True, perf_mode=DR)
            nc.scalar.activation(
                out=v8[:, 2 * up:2 * up + 2, :],
                in_=pv[:, 0:2 * C].rearrange("p (i c) -> p i c", i=2),
                func=AF.Copy, scale=1.0 / 4.0)
        st.update(q_hi=q_hi, q_lo=q_lo, k_hi=k_hi, k_lo=k_lo, v8=v8)
        del st["y_keep"]

    def phase2(g):
        st = state[g]
        q_hi, q_lo = st["q_hi"], st["q_lo"]
        k_hi, k_lo = st["k_hi"], st["k_lo"]
        v8 = st["v8"]

        attn8 = sb2.tile([128, 4, NH, 64], F8, tag="attn8")
        for u in range(4):
            sc = ps4.tile([128, NH, 64], F32, tag="p", name=f"sc_{g}_{u}")
            nc.tensor.matmul(sc, idf_sb,
                             bias_sb.rearrange("m s h j -> m s (h j)"),
                             start=True, stop=False, perf_mode=DR)
            for h in range(NH):
                if h < 4:
                    k_sl, q_sl, off = k_hi, q_hi, 32 * h
                else:
                    k_sl, q_sl, off = k_lo, q_lo, 32 * (h - 4)
                c0 = 128 * u
                nc.tensor.matmul(
                    sc[0:64, h, :],
                    k_sl[off:off + 32, c0:c0 + 64],
                    q_sl[off:off + 32, c0:c0 + 64],
                    start=False, stop=False,
                    tile_position=(off, 0), skip_group_check=True)
                nc.tensor.matmul(
                    sc[64:128, h, :],
                    k_sl[off:off + 32, c0 + 64:c0 + 128],
                    q_sl[off:off + 32, c0 + 64:c0 + 128],
                    start=False, stop=(h == NH - 1),
                    tile_position=(off, 64), skip_group_check=True)
            nc.scalar.activation(out=attn8[:, u, :, :],
                                 in_=sc.rearrange("p h j -> p (h j)"),
                                 func=AF.Exp, bias=ln16_sb)

        # rowsums -> [12, 256] (row 4m+2s+w), one accumulated chain
        r12 = ps4.tile([12, 256], F32, tag="p", name=f"r12_{g}")
        for h in range(NH):
            nc.tensor.matmul(r12.rearrange("p (u j) -> p u j", u=4),
                             eh_sb[:, h, :], attn8[:, :, h, :],
                             start=(h == 0), stop=(h == NH - 1))
        r12_sb = sb3.tile([12, 256], BF16, tag="r12_sb")
        with nc.allow_low_precision(reason="softmax 1/sum in bf16"):
            nc.vector.reciprocal(out=r12_sb, in_=r12)
        rbc = [sb3.tile([96, 2, 4, 64], BF16, tag=f"rbc{w}", name=f"rbc{w}_{g}")
               for w in range(2)]
        for s in range(2):
            for w in range(2):
                r0 = 3 * (2 * s + w)
                nc.sync.dma_start(
                    out=rbc[w][:, s, :, :],
                    in_=r12_sb[r0:r0 + 3, :].unsqueeze(1)
                        .broadcast_to([3, 32, 256]))

        # AV into [96, 2, 512] 2-bank psum; head h=(s,m) at rows 32m, sub s
        ao = ps_ao.tile([96, 2, 512], F32, tag="ao")
        for u in range(4):
            for h in range(NH):
                s, m = h // 3, h % 3
                for w in range(2):
                    nc.tensor.matmul(
                        ao[32 * m:32 * m + 32, s,
                           128 * u + 64 * w:128 * u + 64 * w + 64],
                        v8[64 * w:64 * w + 64, u, 32 * h:32 * h + 32],
                        attn8[64 * w:64 * w + 64, u, h, :],
                        start=True, stop=True,
                        tile_position=(64 * w, 32 * m))
        aoT8 = sbG.tile([96, 2, 512], F8, tag="aoT8")
        ao_v = ao.rearrange("p s (u w j) -> p s u w j", u=4, w=2)
        aoT_v = aoT8.rearrange("p s (u w j) -> p s u w j", u=4, w=2)
        with nc.allow_low_precision(reason="attn out fp8"):
            for w in range(2):
                nc.vector.scalar_tensor_tensor(
                    out=aoT_v[:, :, :, w, :], in0=ao_v[:, :, :, w, :],
                    scalar=1.0 / 16.0, in1=rbc[w],
                    op0=ALU.mult, op1=ALU.mult)
        st["aoT8"] = aoT8
        for kk in ("q_hi", "q_lo", "k_hi", "k_lo", "v8"):
            del st[kk]

    def phase3a(g, mvb2):
        gi = g % BATCH
        st = state[g]
        aoT8 = st["aoT8"]
        x_keep = st["x_keep"]
        # proj TOKEN-major: out[tok, c] per u-pair psum, fused residual
        x2 = sbG.tile([128, 4, C], F32, tag="x2")
        for up in range(2):
            pp = ps4.tile([128, 512], F32, tag="p")
            for i in range(2):
                u = 2 * up + i
                nc.tensor.matmul(pp[:, i * C:(i + 1) * C],
                                 aoT8[:, :, 128 * u:128 * u + 128], wp_sb,
                                 start=True, stop=True, perf_mode=DR)
            nc.vector.scalar_tensor_tensor(
                out=x2[:, 2 * up:2 * up + 2, :],
                in0=pp[:, 0:2 * C].rearrange("p (i c) -> p i c", i=2),
                scalar=1.0 / WS8,
                in1=x_keep[:, 2 * up:2 * up + 2, :],
                op0=ALU.mult, op1=ALU.add)
        for u in range(4):
            ln_stats(x2[:, u, :], mvb2[:, gi], u)
        st["x2"] = x2
        for kk in ("x_keep", "aoT8"):
            del st[kk]

    def phase3b(g, mvb2):
        gi = g % BATCH
        st = state[g]
        xn2T = transposed_fold(st["x2"], mvb2[:, gi], 0, "xn2T", "dve")
        st["xn2T"] = xn2T

    def phase4(g):
        st = state[g]
        xn2T = st["xn2T"]
        hT = sb2.tile([128, 6, 512], F8, tag="hT")
        for m in range(6):
            ph = ps4.tile([128, 512], F32, tag="p")
            nc.tensor.matmul(ph, w1_sb[:, :, 128 * m:128 * m + 128], xn2T,
                             start=True, stop=True, perf_mode=DR)
            nc.scalar.activation(out=hT[:, m, :], in_=ph, func=AF.Gelu,
                                 scale=1.0 / WS8)

        x2 = st["x2"]
        o_t = sb2.tile([128, 4, C], F32, tag="o_t")
        for up in range(2):
            p2 = ps4.tile([128, 512], F32, tag="p")
            for i in range(2):
                u = 2 * up + i
                for j in range(3):
                    nc.tensor.matmul(
                        p2[:, i * C:(i + 1) * C],
                        hT[:, 2 * j:2 * j + 2, 128 * u:128 * u + 128],
                        w2_sb[:, j, :, :],
                        start=(j == 0), stop=(j == 2), perf_mode=DR)
            nc.vector.scalar_tensor_tensor(
                out=o_t[:, 2 * up:2 * up + 2, :],
                in0=p2[:, 0:2 * C].rearrange("p (i c) -> p i c", i=2),
                scalar=1.0 / WS8,
                in1=x2[:, 2 * up:2 * up + 2, :],
                op0=ALU.mult, op1=ALU.add)
        nc.sync.dma_start(
            out=out[g * 512:(g + 1) * 512, :].rearrange("(u p) c -> p u c", u=4),
            in_=o_t)
        del state[g]

    sbB = pool("sbB", 2)
    with nc.allow_low_precision(reason="fp8 kernel"):
        mvb_cur = None
        for b0 in range(0, N_GROUPS, BATCH):
            gs = range(b0, b0 + BATCH)
            last = b0 + BATCH >= N_GROUPS
            if b0 == 0:
                mvb_cur = sbB.tile([128, BATCH, 2, 8], F32, tag="mvb",
                                   name="mvb_0")
                for g in gs:
                    load1(g)
                for g in gs:
                    stats1(g, mvb_cur)
            mvb_this = mvb_cur
            mvb2 = sbB.tile([128, BATCH, 2, 4], F32, tag="mvb2")
            hb = BATCH // 2
            ln_finalize_batch(mvb_this, 0)
            for g in list(gs)[:hb]:
                phase1b(g, mvb_this)
            ln_finalize_batch(mvb_this, 1)
            for g in list(gs)[hb:]:
                phase1b(g, mvb_this)
            for g in gs:
                phase2(g)
            for g in gs:
                phase3a(g, mvb2)
            if not last:
                # prefetch next batch's x/y while P3b/P4 of this batch run
                for g in gs:
                    load1(g + BATCH)
                mvb_cur = sbB.tile([128, BATCH, 2, 8], F32, tag="mvb",
                                   name=f"mvb_{b0 + BATCH}")
            ln_finalize_batch(mvb2, 0)
            for g in list(gs)[:hb]:
                phase3b(g, mvb2)
            ln_finalize_batch(mvb2, 1)
            for g in list(gs)[hb:]:
                phase3b(g, mvb2)
            for g in gs:
                phase4(g)
                if not last:
                    stats1(g + BATCH, mvb_cur)

    for p in reversed(ctx_pools):
        p.__exit__(None, None, None)


def prep_inputs(inputs):
    """Host-side prep: fold norms/scales into weights, build constants."""
    f32 = lambda a: np.ascontiguousarray(np.asarray(a, np.float32))
    x, y = f32(inputs['x']), f32(inputs['y'])
    qkv_w, qkv_b = f32(inputs['qkv_w']), f32(inputs['qkv_b'])
    g1, b1n = f32(inputs['norm1_g']), f32(inputs['norm1_b'])
    g2, b2n = f32(inputs['norm2_g']), f32(inputs['norm2_b'])

    wq_eff = g1[:, None] * qkv_w[:, 0:C] * SCALE
    wk_eff = g1[:, None] * qkv_w[:, C:2 * C]
    wv_eff = g1[:, None] * qkv_w[:, 2 * C:]
    bq = b1n @ qkv_w[:, 0:C] * SCALE + qkv_b[0:C] * SCALE
    bk = b1n @ qkv_w[:, C:2 * C] + qkv_b[C:2 * C]
    bv = b1n @ qkv_w[:, 2 * C:] + qkv_b[2 * C:]
    w1_eff = g2[:, None] * f32(inputs['fc1_w'])
    b1_eff = b2n @ f32(inputs['fc1_w']) + f32(inputs['fc1_b'])
    assert not (np.any(bq) or np.any(bk) or np.any(bv) or np.any(b1_eff) or
                np.any(f32(inputs['proj_b'])) or np.any(f32(inputs['fc2_b']))), \
        "nonzero biases not folded in this build"

    rel = _rel_index()
    rpb = f32(inputs['rpb_table'])
    bias_full = rpb[rel]                     # [n(query), m(key), NH]

    # preload: out[p, (h,j)] = sum_{m,s} idf[m, s, p] * biasR[m, s, h, j]
    #   idf[m, s, p] = 0.25 * (64*s + m == p);  biasR = 4 * bias_full[j, k, h]
    #   -> out = bias_full[j, p % 64, h]  (window-independent)
    biasr = np.zeros((64, 2, NH, 64), np.float32)
    for h in range(NH):
        bh = bias_full[:, :, h]              # [query j, key k]
        for s in range(2):
            biasr[:, s, h, :] = 4.0 * bh.T   # [k(m), j] for keys 64s+m
    idf = np.zeros((64, 2, 128), np.float32)
    for s in range(2):
        for m in range(64):
            idf[m, s, 64 * s + m] = 0.25

    ehab = np.zeros((128, NH, 12), np.float32)
    for h in range(NH):
        s, m = h // 3, h % 3
        for w in range(2):
            ehab[64 * w:64 * w + 64, h, 3 * (2 * s + w) + m] = 1.0

    ident = np.eye(128, dtype=np.float32)

    ebm = np.zeros((12, 4, 96), np.float32)
    for s in range(2):
        for w in range(2):
            for m in range(3):
                ebm[3 * (2 * s + w) + m, 2 * s + w, 32 * m:32 * m + 32] = 1.0

    shared = {
        'wq8': f8(fold96(wq_eff * WS8)),
        'wk8': f8(fold96(wk_eff * WS8)),
        'wv8': f8(fold96(wv_eff * WS8)),
        'wp8': f8(fold96(f32(inputs['proj_w']) * WS8)),
        'w18': f8(fold96(w1_eff * WS8)),
        'w28': f8(np.ascontiguousarray(
            (f32(inputs['fc2_w']) * WS8).reshape(3, 2, 128, C)
            .transpose(2, 0, 1, 3))),
        'biasR8': f8(biasr),
        'idf8': f8(idf),
        'ehAB8': f8(ehab),
        'eB': bf16(ebm),
        'identB': bf16(ident),
    }

    in_maps = []
    for i in range(N_CORES):
        m = dict(shared)
        m['xs'] = win_permute(x[0, i * DS:(i + 1) * DS])
        m['ys'] = win_permute(y[0, i * DS:(i + 1) * DS])
        in_maps.append(m)
    return in_maps


_CACHED_NC = None


def get_program(in_maps=None):
    global _CACHED_NC
    if _CACHED_NC is None:
        _CACHED_NC = build_program()
    return _CACHED_NC


def kernel(**inputs):
    in_maps = prep_inputs(inputs)
    nc = get_program(in_maps)
    res = run_bass_kernel_spmd(nc, in_maps, list(range(N_CORES)))
    outs = [win_unpermute(res.results[i]["out"]) for i in range(N_CORES)]
    full = np.concatenate([o[None] for o in outs], axis=0)  # [8, DS, H, W, C]
    full = full.reshape(1, D, H, W, C).astype(np.float32)
    return full
